# revision 26
# baseline (speedup 1.0000x reference)
"""BitNet attention TRN2 kernel v2: 8-core SPMD (2 batch groups x 4 head groups).

Per core cid = 4*g + j (g = batch index, j = head-group index):
  - host prep: weights are ternarized bit-exactly with the reference's jax
    fp32 formula, scaled, sliced, transposed, and sent as fp16; x inputs are
    hi/lo-split fp16, transposed, and blocked on host (no on-chip transposes,
    no on-chip ternarize, no scale AllReduce).
  - projections: fp16 matmuls, q/k in x-hi + x-lo passes for fp32-grade
    precision, v single-pass; outputs written directly as fp16 hi/lo.
  - attention: per-kb (512-key block) max -> exp pipeline so PSUM banks free
    early; softmax normalization folded into per-partition scalars applied to
    fp16 probs; probs transposed via DMA xbar; attn-mean accumulated fp16
    (host divides by 16); output projection partial summed over the core's
    512 attended dims.
  - per-qb fused [mean | out] fp16 slice ReduceScattered over the 4-core
    batch group.
"""

import os

import numpy as np

os.environ.setdefault("NEURON_RT_RESET_CORES", "1")

B, S, D, H = 2, 2048, 2048, 16
HD = D // H            # 128 head dim
HG = H // 4            # 4 heads per core
OS = HG * HD           # 512-wide slice per core
P = 128
NCORES = 8
NDT = D // P           # 16 contraction tiles
C_SCALE = np.float32(1.0 / np.sqrt(HD))

_CACHE = {}
_last_in_maps = None


def _build(use_mask: bool, zero_bias: bool, pp: int = 2, sp: int = 3,
           single: bool = False, acc_dve: bool = False, rs_skip: bool = False,
           phase_mode: str = "full"):
    import concourse.mybir as mybir
    import concourse.tile as tile
    from concourse import bacc

    F32 = mybir.dt.float32
    F16 = mybir.dt.float16
    I32 = mybir.dt.int32
    AX = mybir.AxisListType
    ALU = mybir.AluOpType
    ACTF = mybir.ActivationFunctionType

    nc = bacc.Bacc("TRN2", target_bir_lowering=False, debug=False,
                   num_devices=1 if single else NCORES)

    def cc(kind, op, groups, ins, outs):
        if rs_skip:
            nc.sync.dma_start(out=outs[0], in_=ins[0][0:outs[0].shape[0]])
            return
        if not single:
            nc.gpsimd.collective_compute(kind, op, replica_groups=groups,
                                         ins=ins, outs=outs)
            return
        src_ap, dst_ap = ins[0], outs[0]
        if kind == "ReduceScatter":
            nc.gpsimd.dma_start(out=dst_ap, in_=src_ap[0:dst_ap.shape[0]])
        else:
            nc.gpsimd.dma_start(out=dst_ap, in_=src_ap)

    # ---- I/O (all per-core slices prepped on host) ----
    x_in = {}
    for nm in ("q", "k", "v"):
        x_in[nm, "h"] = nc.dram_tensor(f"x{nm}h", [4, P, NDT, 512], F16,
                                       kind="ExternalInput")
        if pp == 2 and nm != "v":
            x_in[nm, "l"] = nc.dram_tensor(f"x{nm}l", [4, P, NDT, 512], F16,
                                           kind="ExternalInput")
    w_in = {
        "q": nc.dram_tensor("tq", [P, NDT, OS], F16, kind="ExternalInput"),
        "k": nc.dram_tensor("tk", [P, NDT, OS], F16, kind="ExternalInput"),
        "v": nc.dram_tensor("tv", [P, NDT, OS], F16, kind="ExternalInput"),
        "o": nc.dram_tensor("to", [P, HG, D], F16, kind="ExternalInput"),
    }
    if not zero_bias:
        bq_d = nc.dram_tensor("bq_s", [P, HG], F32, kind="ExternalInput")
        bk_d = nc.dram_tensor("bk_s", [P, HG], F32, kind="ExternalInput")
        bv_d = nc.dram_tensor("bvsc", [P, HG], F32, kind="ExternalInput")
        bo_d = nc.dram_tensor("bo_full", [1, D], F32, kind="ExternalInput")
    if use_mask:
        mask_d = nc.dram_tensor("mask_g", [1, S], I32, kind="ExternalInput")
    # per-qb ReduceScattered outputs (RS writes straight into these)
    mean_d = nc.dram_tensor("mean_slice", [4, P, S], F16,
                            kind="ExternalOutput")
    out_d = nc.dram_tensor("out_slice", [4, P, D], F16,
                           kind="ExternalOutput")

    groups4 = [[0, 1, 2, 3], [4, 5, 6, 7]]

    with tile.TileContext(nc) as tc:
        with tc.tile_pool(name="dram", bufs=1, space="DRAM") as dram, \
             tc.tile_pool(name="const", bufs=1) as const:

            mean_part = [dram.tile([512, S], F16, name=f"mean_part{i}")
                         for i in range(4)]
            out_part = [dram.tile([512, D], F16, name=f"out_part{i}")
                        for i in range(4)]
            mean_rs = [dram.tile([P, S], F16, name=f"mean_rs{i}")
                       for i in range(4)]
            out_rs = [dram.tile([P, D], F16, name=f"out_rs{i}")
                      for i in range(4)]

            if not zero_bias:
                bias_sb = {}
                for nm, d in (("q", bq_d), ("k", bk_d), ("v", bv_d)):
                    t = const.tile([P, HG], F32, name=f"bias_{nm}")
                    nc.sync.dma_start(out=t[:], in_=d.ap()[:])
                    bias_sb[nm] = t
                bo_bc = const.tile([P, D], F32)
                nc.gpsimd.dma_start(out=bo_bc[0:1, :], in_=bo_d.ap()[:])
                nc.gpsimd.partition_broadcast(bo_bc[:], bo_bc[0:1, :])

            # ---------- persistent projection outputs ----------
            with tc.tile_pool(name="kv", bufs=1) as kvp:
                kT_hi = kvp.tile([P, HG, S], F16)        # [d', h, s]
                kT_lo = kvp.tile([P, HG, S], F16)
                qT_hi = kvp.tile([P, HG, S], F16)
                qT_lo = kvp.tile([P, HG, S], F16)
                v_sb = kvp.tile([P, 16, OS], F16)        # [s_p, st, o]

                # ---------- Phase X: projections ----------
                with tc.tile_pool(name="wt", bufs=2) as wtp, \
                     tc.tile_pool(name="xt", bufs=2) as xtp, \
                     tc.tile_pool(name="qstage", bufs=3) as qstg, \
                     tc.tile_pool(name="pmm", bufs=4, space="PSUM") as pmm:

                    for nm in ("q", "k", "v"):
                        wT = wtp.tile([P, NDT, OS], F16, tag="wT",
                                      name=f"wT_{nm}")
                        nc.sync.dma_start(out=wT[:], in_=w_in[nm].ap()[:])
                        for sb in range(4):
                            xTh = xtp.tile([P, NDT, 512], F16, tag="xTh")
                            nc.sync.dma_start(
                                out=xTh[:], in_=x_in[nm, "h"].ap()[sb])
                            xTl = None
                            if pp == 2 and nm != "v":
                                xTl = xtp.tile([P, NDT, 512], F16, tag="xTl")
                                nc.sync.dma_start(
                                    out=xTl[:], in_=x_in[nm, "l"].ap()[sb])
                            if nm == "v":
                                for st_i in range(4):
                                    pv = pmm.tile([P, OS], F32, tag="pp")
                                    stl = slice(st_i * P, (st_i + 1) * P)
                                    for dt_i in range(NDT):
                                        nc.tensor.matmul(
                                            pv[:], xTh[:, dt_i, stl],
                                            wT[:, dt_i, :],
                                            start=(dt_i == 0),
                                            stop=(dt_i == NDT - 1))
                                    nc.scalar.activation(
                                        v_sb[:, sb * 4 + st_i, :], pv[:],
                                        ACTF.Copy)
                            else:
                                th, tl = ((qT_hi, qT_lo) if nm == "q"
                                          else (kT_hi, kT_lo))
                                for ot in range(HG):
                                    pq = pmm.tile([P, 512], F32, tag="pp")
                                    otl = slice(ot * P, (ot + 1) * P)
                                    for dt_i in range(NDT):
                                        nc.tensor.matmul(
                                            pq[:], wT[:, dt_i, otl],
                                            xTh[:, dt_i, :],
                                            start=(dt_i == 0),
                                            stop=(pp == 1
                                                  and dt_i == NDT - 1))
                                        if pp == 2:
                                            nc.tensor.matmul(
                                                pq[:], wT[:, dt_i, otl],
                                                xTl[:, dt_i, :],
                                                start=False,
                                                stop=(dt_i == NDT - 1))
                                    sl5 = slice(sb * 512, (sb + 1) * 512)
                                    if zero_bias:
                                        nc.scalar.activation(
                                            th[:, ot, sl5], pq[:], ACTF.Copy)
                                        nc.vector.tensor_tensor(
                                            out=tl[:, ot, sl5], in0=pq[:],
                                            in1=th[:, ot, sl5],
                                            op=ALU.subtract)
                                    else:
                                        bk = bias_sb[nm][:, ot:ot + 1]
                                        ev = qstg.tile([P, 512], F32,
                                                       tag="ev")
                                        nc.scalar.activation(
                                            ev[:], pq[:], ACTF.Identity,
                                            bias=bk)
                                        nc.scalar.activation(
                                            th[:, ot, sl5], ev[:], ACTF.Copy)
                                        nc.vector.tensor_tensor(
                                            out=tl[:, ot, sl5], in0=ev[:],
                                            in1=th[:, ot, sl5],
                                            op=ALU.subtract)

                # ---------- Phase A: attention + output projection ----------
                with tc.tile_pool(name="wop", bufs=1) as wop, \
                     tc.tile_pool(name="accp", bufs=2) as accp, \
                     tc.tile_pool(name="ptld", bufs=2) as ptld, \
                     tc.tile_pool(name="probs", bufs=3) as probsp, \
                     tc.tile_pool(name="attts", bufs=2) as atttp, \
                     tc.tile_pool(name="smax", bufs=4) as smaxp, \
                     tc.tile_pool(name="outs", bufs=3) as outsp, \
                     tc.tile_pool(name="scp", bufs=2, space="PSUM") as scp, \
                     tc.tile_pool(name="avp", bufs=2, space="PSUM") as avp, \
                     tc.tile_pool(name="pop", bufs=1, space="PSUM") as pop:

                    if phase_mode == "proj":
                        for qb in range(4):
                            nc.sync.dma_start(out=mean_d.ap()[qb],
                                              in_=qT_hi[:, qb, :])
                            nc.sync.dma_start(
                                out=out_d.ap()[qb],
                                in_=v_sb[:, qb * 4:(qb + 1) * 4, :])
                    qb_range = [] if phase_mode == "proj" else range(4)
                    woTb = wop.tile([P, HG, D], F16)  # [d'_p, dt, o]
                    nc.sync.dma_start(out=woTb[:], in_=w_in["o"].ap()[:])
                    if use_mask:
                        mbias = accp.tile([P, S], F32, tag="mbias")
                        nc.gpsimd.dma_start(out=mbias[0:1, :],
                                            in_=mask_d.ap()[:])
                        nc.vector.tensor_scalar(
                            out=mbias[0:1, :], in0=mbias[0:1, :],
                            scalar1=-1.0, scalar2=1e9,
                            op0=ALU.add, op1=ALU.mult)
                        nc.gpsimd.partition_broadcast(mbias[:],
                                                      mbias[0:1, :])
                    for qb in qb_range:
                        acc = accp.tile([P, 4, S], F16, tag="acc")
                        attT_sb = atttp.tile([P, HG, 512], F16, tag="attT")
                        q0c = qb * 512

                        def attended(h, probsT):
                            pav = avp.tile([P, 512], F32, tag="av")
                            for kt in range(16):
                                nc.tensor.matmul(
                                    pav[:], v_sb[:, kt, h * P:(h + 1) * P],
                                    probsT[:, kt, :],
                                    start=(kt == 0), stop=(kt == 15))
                            if zero_bias:
                                nc.scalar.activation(
                                    attT_sb[:, h, :], pav[:], ACTF.Copy)
                            else:
                                nc.scalar.activation(
                                    attT_sb[:, h, :], pav[:], ACTF.Identity,
                                    bias=bias_sb["v"][:, h:h + 1])

                        pT_prev = None
                        for h in range(HG):
                            probsT = ptld.tile([P, 16, 512], F16, tag="pT")
                            for qt in range(4):
                                qcol = slice(q0c + qt * P, q0c + (qt + 1) * P)
                                probs = probsp.tile([P, S], F16, tag="probs")
                                nm2 = smaxp.tile([P, 2], F32, tag="nm2")
                                den2 = smaxp.tile([P, 2], F32, tag="den2")
                                for kp in range(2):
                                    psc = scp.tile([P, 2, 512], F32,
                                                   tag="sc")
                                    for ki in range(2):
                                        kb = kp * 2 + ki
                                        kbs = slice(kb * 512, (kb + 1) * 512)
                                        nc.tensor.matmul(
                                            psc[:, ki, :],
                                            qT_hi[:, h, qcol],
                                            kT_hi[:, h, kbs],
                                            start=True, stop=(sp == 1))
                                        if sp >= 2:
                                            nc.tensor.matmul(
                                                psc[:, ki, :],
                                                qT_lo[:, h, qcol],
                                                kT_hi[:, h, kbs],
                                                start=False, stop=(sp == 2))
                                        if sp >= 3:
                                            nc.tensor.matmul(
                                                psc[:, ki, :],
                                                qT_hi[:, h, qcol],
                                                kT_lo[:, h, kbs],
                                                start=False, stop=True)
                                        if use_mask:
                                            nc.vector.tensor_tensor(
                                                out=psc[:, ki, :],
                                                in0=psc[:, ki, :],
                                                in1=mbias[:, kbs],
                                                op=ALU.add)
                                    # one shared max per 1024-key pair:
                                    # keys >17 below the pair max underflow
                                    # f16 exp, but their true weight is
                                    # < e^-17 -- negligible
                                    nc.vector.tensor_reduce(
                                        out=nm2[:, kp:kp + 1], in_=psc[:],
                                        axis=AX.XY, op=ALU.max, negate=True)
                                    nc.scalar.activation(
                                        probs[:, kp * 1024:(kp + 1) * 1024]
                                        .rearrange("p (a b) -> p a b", a=2),
                                        psc[:], ACTF.Exp,
                                        bias=nm2[:, kp:kp + 1], scale=1.0,
                                        accum_out=den2[:, kp:kp + 1])
                                # M = max_kp m_kp; f_kp = exp(m_kp - M)
                                # den = sum_kp den_kp*f_kp; sc_kp = f_kp/den
                                mneg = smaxp.tile([P, 1], F32, tag="mneg")
                                nc.vector.tensor_reduce(
                                    out=mneg[:], in_=nm2[:], axis=AX.X,
                                    op=ALU.min)
                                f2 = smaxp.tile([P, 2], F32, tag="f2")
                                nc.scalar.activation(
                                    f2[:], nm2[:], ACTF.Exp,
                                    bias=mneg[:], scale=-1.0)
                                tmp2 = smaxp.tile([P, 2], F32, tag="tmp2")
                                dent = smaxp.tile([P, 1], F32, tag="dent")
                                nc.vector.tensor_tensor(
                                    out=tmp2[:], in0=den2[:], in1=f2[:],
                                    op=ALU.mult)
                                nc.vector.tensor_reduce(
                                    out=dent[:], in_=tmp2[:], axis=AX.X,
                                    op=ALU.add)
                                rden = smaxp.tile([P, 1], F32, tag="rden")
                                nc.vector.reciprocal(out=rden[:],
                                                     in_=dent[:])
                                sc2 = smaxp.tile([P, 2], F32, tag="sc2")
                                nc.vector.tensor_scalar(
                                    out=sc2[:], in0=f2[:], scalar1=rden[:],
                                    scalar2=None, op0=ALU.mult)
                                # h==0: scale straight into acc (it doubles
                                # as the first-head accumulator; the
                                # transpose below reads it)
                                for kp in range(2):
                                    kps = slice(kp * 1024, (kp + 1) * 1024)
                                    dst = (acc[:, qt, kps] if h == 0
                                           else probs[:, kps])
                                    nc.vector.tensor_scalar(
                                        out=dst, in0=probs[:, kps],
                                        scalar1=sc2[:, kp:kp + 1],
                                        scalar2=None, op0=ALU.mult)
                                if h > 0:
                                    nc.vector.tensor_tensor(
                                        out=acc[:, qt, :], in0=acc[:, qt, :],
                                        in1=probs[:], op=ALU.add)
                                nc.sync.dma_start_transpose(
                                    out=probsT[:, :, qt * P:(qt + 1) * P],
                                    in_=(acc[:, qt, :] if h == 0
                                         else probs[:]))
                            if phase_mode != "noatt":
                                # one-head software pipeline: attended(h-1)
                                # runs behind scores/probs of head h, hiding
                                # the probs-transpose latency from the PE
                                if pT_prev is not None:
                                    attended(h - 1, pT_prev)
                            pT_prev = probsT
                        if phase_mode != "noatt":
                            attended(HG - 1, pT_prev)
                        # mean partial (sum of 4 heads' probs; host / 16);
                        # RS it now so it overlaps the output projection
                        nc.sync.dma_start(
                            out=mean_part[qb][:]
                            .rearrange("(qt p) k -> p qt k", p=P),
                            in_=acc[:])
                        cc("ReduceScatter", ALU.add, groups4,
                           [mean_part[qb][:]], [mean_rs[qb][:]])
                        # Pool queue: waits on the RS anyway; keeps the SP
                        # FIFO free for the next qb's probs transposes
                        nc.gpsimd.dma_start(out=mean_d.ap()[qb],
                                            in_=mean_rs[qb][:])
                        if phase_mode == "noatt":
                            nc.sync.dma_start(out=out_d.ap()[qb],
                                              in_=probsT[:, 0:4, :])
                            continue
                        if phase_mode == "noout":
                            nc.sync.dma_start(out=out_d.ap()[qb],
                                              in_=attT_sb[:])
                            continue
                        # ---- output projection partial for this qb ----
                        for qt in range(4):
                            osb = outsp.tile([P, D], F16, tag="osb")
                            qtl = slice(qt * P, (qt + 1) * P)
                            for ocp in range(2):
                                po = pop.tile([P, 2, 512], F32, tag="po")
                                for dt in range(HG):
                                    for oc2 in range(2):
                                        oc = ocp * 2 + oc2
                                        ocl = slice(oc * 512,
                                                    (oc + 1) * 512)
                                        nc.tensor.matmul(
                                            po[:, oc2, :],
                                            attT_sb[:, dt, qtl],
                                            woTb[:, dt, ocl],
                                            start=(dt == 0),
                                            stop=(dt == HG - 1))
                                opl = slice(ocp * 1024, (ocp + 1) * 1024)
                                if zero_bias:
                                    nc.scalar.activation(
                                        osb[:, opl], po[:], ACTF.Copy)
                                else:
                                    nc.vector.tensor_tensor(
                                        out=osb[:, opl], in0=po[:],
                                        in1=bo_bc[:, opl], op=ALU.add)
                            nc.sync.dma_start(
                                out=out_part[qb][qt * P:(qt + 1) * P, :],
                                in_=osb[:])
                        cc("ReduceScatter", ALU.add, groups4,
                           [out_part[qb][:]], [out_rs[qb][:]])
                        nc.gpsimd.dma_start(out=out_d.ap()[qb],
                                            in_=out_rs[qb][:])

    nc.compile()
    return nc


def _ternary_like_reference(w):
    """Bit-exact replica of reference.bitnet_weight_ste's quantization
    (jax fp32 on CPU), returning the ternary {-1,0,1} matrix and scale."""
    import jax
    import jax.numpy as jnp
    cpu = jax.devices("cpu")[0]
    with jax.default_device(cpu):
        wj = jnp.asarray(w, jnp.float32)
        scale = jnp.clip(jnp.mean(jnp.abs(wj)), 1e-5, 1000.0)
        wn = jnp.clip(wj / scale, -10.0, 10.0)
        thr = np.float32(2.0 / 3.0)
        wq = jnp.where(wn > thr, 1.0, jnp.where(wn < -thr, -1.0, 0.0))
        return np.asarray(wq, np.float32), np.float32(scale)


def _xT_blocks(x, dtype=np.float16):
    """[S, D] fp32 -> hi/lo fp16 [4, P, NDT, 512] transposed + blocked."""
    xh = x.astype(dtype)
    xl = (x - xh.astype(np.float32)).astype(dtype)

    def blk(a):
        # arr[sb, dp, dt, sc] = a[s, d], d = dt*128+dp, s = sb*512+sc
        return np.ascontiguousarray(
            a.T.reshape(NDT, P, 4, 512).transpose(2, 1, 0, 3))
    return blk(xh), blk(xl)


def _wT_blocks(t_scaled):
    """[OS rows, D] scaled ternary -> [P, NDT, OS] fp16 (transposed)."""
    return np.ascontiguousarray(
        t_scaled.T.reshape(NDT, P, OS).transpose(1, 0, 2)
        .astype(np.float16))


def kernel(**inputs):
    global _last_in_maps
    query = np.ascontiguousarray(inputs["query"], dtype=np.float32)
    key = np.ascontiguousarray(inputs["key"], dtype=np.float32)
    value = np.ascontiguousarray(inputs["value"], dtype=np.float32)
    mask = np.asarray(inputs["mask"])
    bs = {n: np.ascontiguousarray(inputs[n], dtype=np.float32)
          for n in ("bq", "bk", "bv", "bo")}

    tern = {}
    for n in ("wq", "wk", "wv", "wo"):
        tern[n] = _ternary_like_reference(
            np.ascontiguousarray(inputs[n], dtype=np.float32))

    use_mask = not bool(np.all(mask == 1))
    zero_bias = all(bool(np.all(bs[n] == 0)) for n in bs)
    cfg = (use_mask, zero_bias)
    if cfg not in _CACHE:
        _CACHE[cfg] = _build(use_mask, zero_bias)
    nc = _CACHE[cfg]

    # host prep shared across the 4 cores of each batch group
    xq = [None, None]
    xk = [None, None]
    xv = [None, None]
    for g in range(B):
        xq[g] = _xT_blocks(query[g])
        xk[g] = _xT_blocks(key[g])
        xv[g] = _xT_blocks(value[g])

    sq = np.float32(tern["wq"][1] * C_SCALE)
    in_maps = []
    for cid in range(NCORES):
        g, j = divmod(cid, 4)
        sl = slice(OS * j, OS * (j + 1))
        m = {
            "xqh": xq[g][0], "xql": xq[g][1],
            "xkh": xk[g][0], "xkl": xk[g][1],
            "xvh": xv[g][0],
            "tq": _wT_blocks(tern["wq"][0][sl, :] * sq),
            "tk": _wT_blocks(tern["wk"][0][sl, :] * tern["wk"][1]),
            "tv": _wT_blocks(tern["wv"][0][sl, :] * tern["wv"][1]),
            "to": np.ascontiguousarray(
                (tern["wo"][0][:, sl] * tern["wo"][1]).T
                .reshape(HG, P, D).transpose(1, 0, 2).astype(np.float16)),
        }
        if not zero_bias:
            m["bq_s"] = np.ascontiguousarray(
                bs["bq"][sl].reshape(HG, P).T) * C_SCALE  # match q scaling
            m["bk_s"] = np.ascontiguousarray(bs["bk"][sl].reshape(HG, P).T)
            m["bvsc"] = np.ascontiguousarray(bs["bv"][sl].reshape(HG, P).T)
            m["bo_full"] = bs["bo"].reshape(1, D)
        if use_mask:
            m["mask_g"] = np.ascontiguousarray(
                mask[g], dtype=np.int32).reshape(1, S)
        in_maps.append(m)

    _last_in_maps = in_maps

    from concourse.bass_utils import run_bass_kernel_spmd
    res = run_bass_kernel_spmd(nc, in_maps, core_ids=list(range(NCORES)))

    out = np.empty((B, S, D), np.float32)
    attn_mean = np.empty((B, S, S), np.float32)
    for cid in range(NCORES):
        g, j = divmod(cid, 4)
        ms = np.asarray(res.results[cid]["mean_slice"]).astype(np.float32)
        os_ = np.asarray(res.results[cid]["out_slice"]).astype(np.float32)
        for qb in range(4):
            r0 = qb * 512 + P * j
            attn_mean[g][r0:r0 + P, :] = ms[qb] * np.float32(1 / 16)
            out[g][r0:r0 + P, :] = os_[qb]
    return out, attn_mean
